# revision 5
# baseline (speedup 1.0000x reference)
"""DiffEdgeNodeLayer Trainium2 kernel.

Math: reference computes, per (b, o):
    ev_min = min_i(x[b,i]*pe[o,i] + pn[o,i]),  ev_max = max_i(x[b,i]*pe[o,i] - pn[o,i])
    out = ev_min*n0[o] + ev_max*n1[o]
with pe/pn softmax pairs (pn = 1-pe) and n0/n1 softmax pair.

Using pn = 1-pe:
    x*pe + pn = 1 - pe*(1-x)   =>  ev_min = 1 - max_i(pe[o,i]*u[b,i]),  u = 1-x
    x*pe - pn = pe*(1+x) - 1   =>  ev_max = max_i(pe[o,i]*v[b,i]) - 1,  v = 1+x

Both branches are max-over-i of (pe column) * (per-batch scalar).  With
partitions=batch and free=o, each i contributes one fused DVE
scalar_tensor_tensor per (i, b-chunk, branch):
    M = (pe_colT_bcast * u[:, i]) max M        (running max, in place)
pe columns are served by GPSIMD partition_broadcast from partition-0 staged
copies of the transposed PE matrix (TensorE transpose).

Sharding: data-parallel over batch, 8 cores, B=2048 -> 256 rows/core.
"""

import numpy as np

import concourse.bacc as bacc
import concourse.mybir as mybir
import concourse.tile as tile
from concourse._compat import get_trn_type
from concourse.bass_utils import run_bass_kernel_spmd
from concourse.masks import make_identity

N_CORES = 8
B, IN_F, OUT_F = 2048, 256, 256
B_SH = B // N_CORES  # 256 batch rows per core
P = 128  # partitions

F32 = mybir.dt.float32
ALU = mybir.AluOpType
AF = mybir.ActivationFunctionType

_cached_nc = None


def _build():
    nc = bacc.Bacc(
        get_trn_type() or "TRN2",
        target_bir_lowering=False,
        debug=False,
        num_devices=N_CORES,
    )

    x_d = nc.dram_tensor("x", [B_SH, IN_F], F32, kind="ExternalInput")
    pe_d = nc.dram_tensor("pe_w", [OUT_F, IN_F, 2], F32, kind="ExternalInput")
    pn_d = nc.dram_tensor("pn_w", [OUT_F, 2], F32, kind="ExternalInput")
    out_d = nc.dram_tensor("out", [B_SH, OUT_F], F32, kind="ExternalOutput")

    with tile.TileContext(nc) as tc:
        with (
            tc.tile_pool(name="persist", bufs=1) as pp,
            tc.tile_pool(name="rot", bufs=4) as rp,
            tc.tile_pool(name="psum", bufs=4, space="PSUM") as psp,
        ):
            # ---- loads ----
            xt = []
            for c in range(2):
                xc = pp.tile([P, IN_F], F32, tag=f"x{c}", name=f"x{c}")
                nc.sync.dma_start(out=xc[:], in_=x_d.ap()[c * P : (c + 1) * P, :])
                xt.append(xc)
            wt = []
            for t in range(2):
                wtt = pp.tile([P, IN_F, 2], F32, tag=f"w{t}", name=f"w{t}")
                nc.sync.dma_start(out=wtt[:], in_=pe_d.ap()[t * P : (t + 1) * P, :, :])
                wt.append(wtt)
            nrow = pp.tile([1, OUT_F, 2], F32, tag="nrow", name="nrow")
            nc.sync.dma_start(out=nrow[:], in_=pn_d.ap()[:, :])

            # ---- weight prep ----
            # edge prob: pe = sigmoid(w0 - w1), [o_part, i], 2 tiles
            pet = []
            for t in range(2):
                delta = rp.tile([P, IN_F], F32, tag="delta", name="delta")
                nc.vector.tensor_tensor(
                    delta[:], wt[t][:, :, 0], wt[t][:, :, 1], ALU.subtract
                )
                pe_tile = pp.tile([P, IN_F], F32, tag=f"pe{t}", name=f"pe{t}")
                nc.scalar.activation(pe_tile[:], delta[:], AF.Sigmoid)
                pet.append(pe_tile)

            # transpose PE -> PET [i_part, o_free], 2 tiles, via TensorE
            ident = pp.tile([P, P], F32, tag="ident", name="ident")
            make_identity(nc, ident[:])
            pett = []
            for it in range(2):
                pet_t = pp.tile([P, OUT_F], F32, tag=f"pet{it}", name=f"pet{it}")
                for ot in range(2):
                    pst = psp.tile([P, P], F32, tag="pst", name="pst")
                    nc.tensor.transpose(
                        pst[:], pet[ot][:, it * P : (it + 1) * P], ident[:]
                    )
                    nc.scalar.copy(pet_t[:, ot * P : (ot + 1) * P], pst[:])
                pett.append(pet_t)

            # node probs: n0 = sigmoid(d), n1 = 1 - n0, as [1, OUT_F] rows
            ndelta = pp.tile([1, OUT_F], F32, tag="ndelta", name="ndelta")
            nc.vector.tensor_tensor(
                ndelta[:], nrow[:, :, 0], nrow[:, :, 1], ALU.subtract
            )
            n0 = pp.tile([1, OUT_F], F32, tag="n0", name="n0")
            nc.scalar.activation(n0[:], ndelta[:], AF.Sigmoid)
            n1 = pp.tile([1, OUT_F], F32, tag="n1", name="n1")
            nc.vector.tensor_scalar(n1[:], n0[:], -1.0, 1.0, ALU.mult, ALU.add)

            n0b = pp.tile([P, OUT_F], F32, tag="n0b", name="n0b")
            nc.gpsimd.partition_broadcast(n0b[:], n0[:])
            n1b = pp.tile([P, OUT_F], F32, tag="n1b", name="n1b")
            nc.gpsimd.partition_broadcast(n1b[:], n1[:])
            cb = pp.tile([P, OUT_F], F32, tag="cb", name="cb")
            nc.vector.tensor_tensor(cb[:], n0b[:], n1b[:], ALU.subtract)

            # u = 1 - x, v = 1 + x  [b_part, i_free]
            ut, vt = [], []
            for c in range(2):
                uc = pp.tile([P, IN_F], F32, tag=f"u{c}", name=f"u{c}")
                nc.vector.tensor_scalar(uc[:], xt[c][:], -1.0, 1.0, ALU.mult, ALU.add)
                vc = pp.tile([P, IN_F], F32, tag=f"v{c}", name=f"v{c}")
                nc.vector.tensor_scalar_add(vc[:], xt[c][:], 1.0)
                ut.append(uc)
                vt.append(vc)

            # running-max accumulators [b_part, o_free]; products > 0 so 0-init
            m1, m2 = [], []
            for c in range(2):
                m1c = pp.tile([P, OUT_F], F32, tag=f"m1_{c}", name=f"m1_{c}")
                nc.vector.memset(m1c[:], 0.0)
                m1.append(m1c)
                m2c = pp.tile([P, OUT_F], F32, tag=f"m2_{c}", name=f"m2_{c}")
                nc.vector.memset(m2c[:], 0.0)
                m2.append(m2c)

            # ---- main loop over contraction index i ----
            # stage PET rows (pe columns) into partition-0 tiles, 32 at a time
            QROWS = 32
            stages = {}
            for q in range(IN_F // QROWS):
                it = (q * QROWS) // P
                r0 = (q * QROWS) % P
                stage = rp.tile(
                    [1, QROWS, OUT_F], F32, tag="stage", bufs=2, name="stage"
                )
                nc.sync.dma_start(out=stage[:], in_=pett[it][r0 : r0 + QROWS, :])
                stages[q] = stage

            for i in range(IN_F):
                q, i_local = divmod(i, QROWS)
                peb = rp.tile([P, OUT_F], F32, tag="peb", name="peb")
                nc.gpsimd.partition_broadcast(peb[:], stages[q][0:1, i_local, :])
                for c in range(2):
                    nc.vector.scalar_tensor_tensor(
                        m1[c][:], peb[:], ut[c][:, i : i + 1], m1[c][:],
                        ALU.mult, ALU.max,
                    )
                    nc.vector.scalar_tensor_tensor(
                        m2[c][:], peb[:], vt[c][:, i : i + 1], m2[c][:],
                        ALU.mult, ALU.max,
                    )

            # ---- combine: out = (n0-n1) - n0*M1 + n1*M2 ----
            for c in range(2):
                s1 = rp.tile([P, OUT_F], F32, tag="s1", name="s1")
                nc.vector.scalar_tensor_tensor(
                    s1[:], m1[c][:], -1.0, n0b[:], ALU.mult, ALU.mult
                )
                s2 = rp.tile([P, OUT_F], F32, tag="s2", name="s2")
                nc.vector.tensor_tensor(s2[:], m2[c][:], n1b[:], ALU.mult)
                s3 = rp.tile([P, OUT_F], F32, tag="s3", name="s3")
                nc.vector.tensor_tensor(s3[:], s1[:], s2[:], ALU.add)
                oc = rp.tile([P, OUT_F], F32, tag="oc", name="oc")
                nc.vector.tensor_tensor(oc[:], s3[:], cb[:], ALU.add)
                nc.sync.dma_start(out=out_d.ap()[c * P : (c + 1) * P, :], in_=oc[:])

    nc.compile()
    return nc


def _get_nc():
    global _cached_nc
    if _cached_nc is None:
        _cached_nc = _build()
    return _cached_nc


def _make_in_maps(x, pe, pn):
    return [
        {
            "x": np.ascontiguousarray(x[i * B_SH : (i + 1) * B_SH]),
            "pe_w": pe,
            "pn_w": pn,
        }
        for i in range(N_CORES)
    ]


def run(x, prob_edge_weights, prob_node_weights, **spmd_kwargs):
    """Run on hardware; returns (out, BassKernelResults)."""
    nc = _get_nc()
    x = np.ascontiguousarray(np.asarray(x, dtype=np.float32))
    pe = np.ascontiguousarray(np.asarray(prob_edge_weights, dtype=np.float32))
    pn = np.ascontiguousarray(np.asarray(prob_node_weights, dtype=np.float32))
    res = run_bass_kernel_spmd(
        nc, _make_in_maps(x, pe, pn), list(range(N_CORES)), **spmd_kwargs
    )
    out = np.concatenate(
        [res.results[i]["out"] for i in range(N_CORES)], axis=0
    ).astype(np.float32)
    return out, res


def kernel(x, prob_edge_weights, prob_node_weights):
    out, _ = run(x, prob_edge_weights, prob_node_weights)
    return out


# revision 7
# speedup vs baseline: 634.3189x; 634.3189x over previous
"""DiffEdgeNodeLayer Trainium2 kernel.

Math: reference computes, per (b, o):
    ev_min = min_i(x[b,i]*pe[o,i] + pn[o,i]),  ev_max = max_i(x[b,i]*pe[o,i] - pn[o,i])
    out = ev_min*n0[o] + ev_max*n1[o]
with pe/pn softmax pairs (pn = 1-pe) and n0/n1 softmax pair.

Using pn = 1-pe:
    x*pe + pn = 1 - pe*(1-x)   =>  ev_min = 1 - max_i(pe[o,i]*u[b,i]),  u = 1-x
    x*pe - pn = pe*(1+x) - 1   =>  ev_max = max_i(pe[o,i]*v[b,i]) - 1,  v = 1+x

Both branches are max-over-i of (pe column) * (per-batch scalar).  With
partitions=batch and free=o, each i contributes one fused DVE
scalar_tensor_tensor per (i, b-chunk, branch):
    M = (pe_colT_bcast * u[:, i]) max M        (running max, in place)
pe columns are served by GPSIMD partition_broadcast from partition-0 staged
copies of the transposed PE matrix (TensorE transpose).

Sharding: data-parallel over batch, 8 cores, B=2048 -> 256 rows/core.
"""

import numpy as np

import concourse.bacc as bacc
import concourse.mybir as mybir
import concourse.tile as tile
from concourse._compat import get_trn_type
from concourse.bass_utils import run_bass_kernel_spmd
from concourse.masks import make_identity

N_CORES = 8
B, IN_F, OUT_F = 2048, 256, 256
B_SH = B // N_CORES  # 256 batch rows per core
P = 128  # partitions

F32 = mybir.dt.float32
ALU = mybir.AluOpType
AF = mybir.ActivationFunctionType

_cached_nc = None


def _build():
    nc = bacc.Bacc(
        get_trn_type() or "TRN2",
        target_bir_lowering=False,
        debug=False,
        num_devices=N_CORES,
    )

    x_d = nc.dram_tensor("x", [B_SH, IN_F], F32, kind="ExternalInput")
    pe_d = nc.dram_tensor("pe_w", [OUT_F, IN_F, 2], F32, kind="ExternalInput")
    pn_d = nc.dram_tensor("pn_w", [OUT_F, 2], F32, kind="ExternalInput")
    out_d = nc.dram_tensor("out", [B_SH, OUT_F], F32, kind="ExternalOutput")

    with tile.TileContext(nc) as tc:
        with (
            tc.tile_pool(name="persist", bufs=1) as pp,
            tc.tile_pool(name="rot", bufs=4) as rp,
            tc.tile_pool(name="psum", bufs=4, space="PSUM") as psp,
        ):
            # ---- loads ----
            xt = []
            for c in range(2):
                xc = pp.tile([P, IN_F], F32, tag=f"x{c}", name=f"x{c}")
                nc.sync.dma_start(out=xc[:], in_=x_d.ap()[c * P : (c + 1) * P, :])
                xt.append(xc)
            wt = []
            for t in range(2):
                wtt = pp.tile([P, IN_F, 2], F32, tag=f"w{t}", name=f"w{t}")
                nc.sync.dma_start(out=wtt[:], in_=pe_d.ap()[t * P : (t + 1) * P, :, :])
                wt.append(wtt)
            nrow = pp.tile([1, OUT_F, 2], F32, tag="nrow", name="nrow")
            nc.sync.dma_start(out=nrow[:], in_=pn_d.ap()[:, :])

            # ---- weight prep ----
            # edge prob: pe = sigmoid(w0 - w1), [o_part, i], 2 tiles
            pet = []
            for t in range(2):
                delta = rp.tile([P, IN_F], F32, tag="delta", name="delta")
                nc.vector.tensor_tensor(
                    delta[:], wt[t][:, :, 0], wt[t][:, :, 1], ALU.subtract
                )
                pe_tile = pp.tile([P, IN_F], F32, tag=f"pe{t}", name=f"pe{t}")
                nc.scalar.activation(pe_tile[:], delta[:], AF.Sigmoid)
                pet.append(pe_tile)

            # transpose PE -> PET [i_part, o_free], 2 tiles, via TensorE
            ident = pp.tile([P, P], F32, tag="ident", name="ident")
            make_identity(nc, ident[:])
            pett = []
            for it in range(2):
                pet_t = pp.tile([P, OUT_F], F32, tag=f"pet{it}", name=f"pet{it}")
                for ot in range(2):
                    pst = psp.tile([P, P], F32, tag="pst", name="pst")
                    nc.tensor.transpose(
                        pst[:], pet[ot][:, it * P : (it + 1) * P], ident[:]
                    )
                    nc.scalar.copy(pet_t[:, ot * P : (ot + 1) * P], pst[:])
                pett.append(pet_t)

            # node probs: n0 = sigmoid(d), n1 = 1 - n0, as [1, OUT_F] rows
            ndelta = pp.tile([1, OUT_F], F32, tag="ndelta", name="ndelta")
            nc.vector.tensor_tensor(
                ndelta[:], nrow[:, :, 0], nrow[:, :, 1], ALU.subtract
            )
            n0 = pp.tile([1, OUT_F], F32, tag="n0", name="n0")
            nc.scalar.activation(n0[:], ndelta[:], AF.Sigmoid)
            n1 = pp.tile([1, OUT_F], F32, tag="n1", name="n1")
            nc.vector.tensor_scalar(n1[:], n0[:], -1.0, 1.0, ALU.mult, ALU.add)

            n0b = pp.tile([P, OUT_F], F32, tag="n0b", name="n0b")
            nc.gpsimd.partition_broadcast(n0b[:], n0[:])
            n1b = pp.tile([P, OUT_F], F32, tag="n1b", name="n1b")
            nc.gpsimd.partition_broadcast(n1b[:], n1[:])
            cb = pp.tile([P, OUT_F], F32, tag="cb", name="cb")
            nc.vector.tensor_tensor(cb[:], n0b[:], n1b[:], ALU.subtract)

            # u = 1 - x, v = 1 + x  [b_part, i_free]
            ut, vt = [], []
            for c in range(2):
                uc = pp.tile([P, IN_F], F32, tag=f"u{c}", name=f"u{c}")
                nc.vector.tensor_scalar(uc[:], xt[c][:], -1.0, 1.0, ALU.mult, ALU.add)
                vc = pp.tile([P, IN_F], F32, tag=f"v{c}", name=f"v{c}")
                nc.vector.tensor_scalar_add(vc[:], xt[c][:], 1.0)
                ut.append(uc)
                vt.append(vc)

            # running-max accumulators [b_part, o_free]; products > 0 so 0-init
            m1, m2 = [], []
            for c in range(2):
                m1c = pp.tile([P, OUT_F], F32, tag=f"m1_{c}", name=f"m1_{c}")
                nc.vector.memset(m1c[:], 0.0)
                m1.append(m1c)
                m2c = pp.tile([P, OUT_F], F32, tag=f"m2_{c}", name=f"m2_{c}")
                nc.vector.memset(m2c[:], 0.0)
                m2.append(m2c)

            # ---- main loop over contraction index i ----
            import contextlib
            import os

            _repeat = int(os.environ.get("KERNEL_REPEAT", "1"))
            loop_ctx = (
                tc.For_i(0, _repeat, 1) if _repeat > 1 else contextlib.nullcontext()
            )
            with loop_ctx:
                # stage PET rows (pe columns) into partition-0 tiles
                QROWS = 32
                stages = {}
                for q in range(IN_F // QROWS):
                    it = (q * QROWS) // P
                    r0 = (q * QROWS) % P
                    stage = rp.tile(
                        [1, QROWS, OUT_F], F32, tag="stage", bufs=2, name="stage"
                    )
                    nc.sync.dma_start(
                        out=stage[:], in_=pett[it][r0 : r0 + QROWS, :]
                    )
                    stages[q] = stage

                for i in range(IN_F):
                    q, i_local = divmod(i, QROWS)
                    peb = rp.tile([P, OUT_F], F32, tag="peb", name="peb")
                    nc.gpsimd.partition_broadcast(
                        peb[:], stages[q][0:1, i_local, :]
                    )
                    if i % 4 != 3:
                        # ScalarE computes the products; DVE only maxes.
                        for c in range(2):
                            pr1 = rp.tile(
                                [P, OUT_F], F32, tag="pr1", bufs=6, name="pr1"
                            )
                            nc.scalar.activation(
                                pr1[:], peb[:], AF.Copy,
                                bias=0.0, scale=ut[c][:, i : i + 1],
                            )
                            nc.vector.tensor_tensor(
                                m1[c][:], pr1[:], m1[c][:], ALU.max
                            )
                            pr2 = rp.tile(
                                [P, OUT_F], F32, tag="pr2", bufs=6, name="pr2"
                            )
                            nc.scalar.activation(
                                pr2[:], peb[:], AF.Copy,
                                bias=0.0, scale=vt[c][:, i : i + 1],
                            )
                            nc.vector.tensor_tensor(
                                m2[c][:], pr2[:], m2[c][:], ALU.max
                            )
                    else:
                        for c in range(2):
                            nc.vector.scalar_tensor_tensor(
                                m1[c][:], peb[:], ut[c][:, i : i + 1], m1[c][:],
                                ALU.mult, ALU.max,
                            )
                            nc.vector.scalar_tensor_tensor(
                                m2[c][:], peb[:], vt[c][:, i : i + 1], m2[c][:],
                                ALU.mult, ALU.max,
                            )

            # ---- combine: out = (n0-n1) - n0*M1 + n1*M2 ----
            for c in range(2):
                s1 = rp.tile([P, OUT_F], F32, tag="s1", name="s1")
                nc.vector.scalar_tensor_tensor(
                    s1[:], m1[c][:], -1.0, n0b[:], ALU.mult, ALU.mult
                )
                s2 = rp.tile([P, OUT_F], F32, tag="s2", name="s2")
                nc.vector.tensor_tensor(s2[:], m2[c][:], n1b[:], ALU.mult)
                s3 = rp.tile([P, OUT_F], F32, tag="s3", name="s3")
                nc.vector.tensor_tensor(s3[:], s1[:], s2[:], ALU.add)
                oc = rp.tile([P, OUT_F], F32, tag="oc", name="oc")
                nc.vector.tensor_tensor(oc[:], s3[:], cb[:], ALU.add)
                nc.sync.dma_start(out=out_d.ap()[c * P : (c + 1) * P, :], in_=oc[:])

    nc.compile()
    return nc


def _get_nc():
    global _cached_nc
    if _cached_nc is None:
        _cached_nc = _build()
    return _cached_nc


def _make_in_maps(x, pe, pn):
    return [
        {
            "x": np.ascontiguousarray(x[i * B_SH : (i + 1) * B_SH]),
            "pe_w": pe,
            "pn_w": pn,
        }
        for i in range(N_CORES)
    ]


def run(x, prob_edge_weights, prob_node_weights, **spmd_kwargs):
    """Run on hardware; returns (out, BassKernelResults)."""
    nc = _get_nc()
    x = np.ascontiguousarray(np.asarray(x, dtype=np.float32))
    pe = np.ascontiguousarray(np.asarray(prob_edge_weights, dtype=np.float32))
    pn = np.ascontiguousarray(np.asarray(prob_node_weights, dtype=np.float32))
    res = run_bass_kernel_spmd(
        nc, _make_in_maps(x, pe, pn), list(range(N_CORES)), **spmd_kwargs
    )
    out = np.concatenate(
        [res.results[i]["out"] for i in range(N_CORES)], axis=0
    ).astype(np.float32)
    return out, res


def kernel(x, prob_edge_weights, prob_node_weights):
    out, _ = run(x, prob_edge_weights, prob_node_weights)
    return out
